# revision 10
# baseline (speedup 1.0000x reference)
"""AffineLabelAttention Trainium2 kernel (v3).

out[b, l, i, j] = W_h[l] @ head[b, i] + W_d[l] @ dep[b, j] + bias[l]

Shapes (hardcoded): head/dep [4, 1024, 768] f32, label_W [32, 1536], label_b [32].
Full output [4, 32, 1024, 1024] f32 (512 MB) -> completely output-DMA-bound.

Sharding over 8 cores: core c handles batch b = c // 2 and label half
lh = c % 2 (16 labels).

The device stores the output in float16 (pointwise rel err <= 2^-11);
the host upcasts during the unshard. 32 MB of output per core is the
whole cost: a single HWDGE queue fans each DMA across all 16 SDMA
engines and sustains ~420 GB/s (measured, near the 435 GB/s SBUF-AXI
fabric ceiling), so one queue IS the roofline. The kernel's only job
is to start that stream as early as possible and never let it starve.

v3 structure (109 us v2 -> target ~99 us):
  1. Inputs are host-cast to f16 AND host-rearranged so every DMA is
     contiguous per partition ([p][jc][k][s] layout): input staging
     runs at line rate instead of ~320 GB/s.
  2. Staging order per ring: dep before head, head j-half 0 before
     j-half 1, split across both HWDGE rings; consts on SWDGE.
  3. PE warm-up chain sized to end right when dep lands, so the HAM
     clock boost (needs ~4 us of sustained duty) arrives BEFORE the
     score matmuls instead of 8 us into them (v2's first-DMA was 27 us
     because the d/h matmuls ran at half clock).
  4. ALL 128 output adds run on DVE (f16 SBUF tensor_scalar = 4x perf
     mode, ~330 ns per [128,1024] tile; ~56% occupancy under the
     stream). ACT only evacuates PSUM (d/h scores, broadcast rows).
  5. Per label one 2 MB DMA on the sync ring (label 0 split 1 MB +
     1 MB, with the ic4-7 adds emitted after the h j-half-1 transposes
     so the first 1 MB launches without waiting on them).

  Notes baked into the structure:
  - walrus/bass: compute-engine operands must start at partition
    0/32/64 (96 is rejected); engines cannot move data across
    partitions (only PE matmul/transpose and DMA can).
  - PSUM is 8 banks x 2KB: score pool 2 + warmup/transpose pool 2 +
    broadcast pool 4.
  - A DMA trigger that cannot get a ring slot stalls its issuing
    engine: ACT issues only 3 input transfers, all before its compute.
  - PSUM operands cap DVE perf modes, so broadcasts are evacuated to
    SBUF f16 by ACT (closest to PSUM) and the adds read SBUF at 4x.
"""

import sys

import numpy as np

if "/opt/trn_rl_repo" not in sys.path:
    sys.path.insert(0, "/opt/trn_rl_repo")

import concourse.bass as bass
import concourse.mybir as mybir
from concourse import bacc
from concourse.bass_utils import run_bass_kernel_spmd
from concourse.tile import TileContext, add_dep_helper

B, S, D, L = 4, 1024, 768, 32
NCORES = 8
LH = L // 2          # labels per core (16)
KCH = D // 128       # contraction chunks (6)
ICH = S // 128       # i chunks (8)
F32 = mybir.dt.float32
F16 = mybir.dt.float16
WU_N = 8             # PE warm-up matmuls before the score streams

# knobs for test harness
TRACE = False
TRACE_CORES = None
LAST_RESULTS = None

_CACHE = {}


def _build():
    nc = bacc.Bacc("TRN2", target_bir_lowering=False, debug=False)
    # inputs pre-rearranged on host: [partition, jc, k, s'] where
    # d = k*128 + p contracts and j (or i) = jc*512 + s'
    headT = nc.dram_tensor("headT", [128, 2, KCH, 512], F16,
                           kind="ExternalInput")
    depT = nc.dram_tensor("depT", [128, 2, KCH, 512], F16,
                          kind="ExternalInput")
    # packed label weights: cols 0:16 = W_h slice, 16:32 = W_d slice
    wT = nc.dram_tensor("wT", [D, 2 * LH], F16, kind="ExternalInput")
    # bias replicated at partition groups 0 and 32 (one per j-half)
    bcol = nc.dram_tensor("bcol", [48, 1], F32, kind="ExternalInput")
    # one-hot row selectors, replicated at partition groups 0 and 32
    sel = nc.dram_tensor("sel", [48, LH * 128], F16, kind="ExternalInput")
    # identity block for h transposes at partition group 64
    idm = nc.dram_tensor("idm", [80, LH], F32, kind="ExternalInput")
    # [l, p, c, j]: row i = c*128 + p of label l lives at out[l, p, c, :]
    out = nc.dram_tensor("out", [LH, 128, ICH, S], F16, kind="ExternalOutput")
    out_v = out[:]

    headT_f = headT[:]
    depT_f = depT[:]
    wT_f = wT[:].rearrange("(k p) l -> p k l", p=128)         # [128, 6, 32]

    with TileContext(nc) as tc:
        with (
            tc.tile_pool(name="const", bufs=1) as cpool,
            tc.tile_pool(name="outp", bufs=4) as opool,
            tc.tile_pool(name="bcast", bufs=16) as bpool,
            tc.tile_pool(name="psum_sc", bufs=2, space="PSUM") as psc,
            tc.tile_pool(name="psum_tp", bufs=2, space="PSUM") as ptp,
            tc.tile_pool(name="psum_bc", bufs=4, space="PSUM") as pbc,
        ):
            depT_sb = cpool.tile([128, 2, KCH, 512], F16)
            headT_sb = cpool.tile([128, 2, KCH, 512], F16)
            wT_sb = cpool.tile([128, KCH, 2 * LH], F16)
            b_col = cpool.tile([48, 1], F32)
            sel_sb = cpool.tile([48, LH * 128], F16)
            id_sb = cpool.tile([80, LH], F32)
            h_lT = cpool.tile([128, S], F32)     # h scores [l, i] @ parts 64:80
            h_all = cpool.tile([128, ICH, LH], F32)  # h scores, [i, l] layout
            d_sb = cpool.tile([48, S], F16)      # d+bias: jc0 @ 0:16, jc1 @ 32:48
            wu_w = cpool.tile([128, LH], F16)    # PE warm-up operands
            wu_x = cpool.tile([128, 512], F16)

            # Warm-up operand memsets first so DVE clears them at t~0 and
            # the PE warm-up chain starts immediately.
            nc.vector.memset(wu_w[:], 0.0)
            nc.vector.memset(wu_x[:], 0.0)

            # --- input staging -------------------------------------------
            # sync ring:   wT, dep-jc0, head-jc0 k0-2, head-jc1 k0-2,
            #              then all output DMAs (strict FIFO per ring).
            # scalar ring: dep-jc1, head-jc0 k3-5, head-jc1 k3-5 (ACT
            #              issues these before any of its compute).
            # gpsimd ring: consts (sel/id/bias) via SWDGE.
            nc.sync.dma_start(out=wT_sb[:], in_=wT_f[:])
            nc.sync.dma_start(out=depT_sb[:, 0], in_=depT_f[:, 0])
            nc.scalar.dma_start(out=depT_sb[:, 1], in_=depT_f[:, 1])
            nc.sync.dma_start(out=headT_sb[:, 0, 0:3], in_=headT_f[:, 0, 0:3])
            nc.scalar.dma_start(out=headT_sb[:, 0, 3:6],
                                in_=headT_f[:, 0, 3:6])
            nc.sync.dma_start(out=headT_sb[:, 1, 0:3], in_=headT_f[:, 1, 0:3])
            nc.scalar.dma_start(out=headT_sb[:, 1, 3:6],
                                in_=headT_f[:, 1, 3:6])
            nc.gpsimd.dma_start(out=sel_sb[:], in_=sel[:])
            nc.gpsimd.dma_start(out=id_sb[:], in_=idm[:])
            nc.gpsimd.dma_start(out=b_col[:], in_=bcol[:])

            # PE warm-up (builds HAM clock duty while inputs stream in;
            # sized to end about when dep lands). Lives in the transpose
            # pool so it never blocks the score psums.
            wu_ps = ptp.tile([128, 512], F32, name="wu", tag="tp")
            for _ in range(WU_N):
                nc.tensor.matmul(wu_ps[0:LH, :], wu_w[:], wu_x[:],
                                 start=True, stop=True)

            # d scores: two concurrent column-group streams (jc0 @ group 0,
            # jc1 @ group 32), issue-interleaved so the array pipelines the
            # LdWeights of one group under the matmul of the other.
            sc_a = psc.tile([128, 512], F32, name="sc_a", tag="score")
            sc_b = psc.tile([128, 512], F32, name="sc_b", tag="score")
            for k in range(KCH):
                nc.tensor.matmul(
                    sc_a[0:LH, :], wT_sb[:, k, LH:2 * LH],
                    depT_sb[:, 0, k, :],
                    start=(k == 0), stop=(k == KCH - 1),
                    tile_position=(0, 0),
                )
                nc.tensor.matmul(
                    sc_b[32:32 + LH, :], wT_sb[:, k, LH:2 * LH],
                    depT_sb[:, 1, k, :],
                    start=(k == 0), stop=(k == KCH - 1),
                    tile_position=(0, 32),
                )

            # d evacuation (+bias) on ACT (fastest PSUM reader), f16 out
            nc.scalar.add(d_sb[0:LH, 0:512], sc_a[0:LH, :], b_col[0:LH, :])
            nc.scalar.add(d_sb[32:32 + LH, 512:1024],
                          sc_b[32:32 + LH, :], b_col[32:32 + LH, :])

            # h j-half 0 @ group 64 (needs head-jc0 only)
            sc_c = psc.tile([128, 512], F32, name="sc_c", tag="score")
            for k in range(KCH):
                nc.tensor.matmul(
                    sc_c[64:64 + LH, :], wT_sb[:, k, 0:LH],
                    headT_sb[:, 0, k, :],
                    start=(k == 0), stop=(k == KCH - 1),
                    tile_position=(0, 64),
                )
            nc.scalar.copy(h_lT[64:64 + LH, 0:512], sc_c[64:64 + LH, :])

            dbcs = {}

            def bcast(lb):
                # replicate d row lb across 128 partitions: one-hot selector
                # matmuls (f16 exact). Result evacuated to a persistent f16
                # SBUF tile so the adds run in DVE 4x mode.
                dbc = bpool.tile([128, S], F16, name="dbc", tag="dbc")
                for jc in range(2):
                    p0 = 32 * jc
                    bc_ps = pbc.tile([128, 512], F32, name="bc", tag="bc")
                    nc.tensor.matmul(
                        bc_ps[:],
                        sel_sb[p0:p0 + LH, lb * 128:(lb + 1) * 128],
                        d_sb[p0:p0 + LH, jc * 512:(jc + 1) * 512],
                        start=True, stop=True,
                    )
                    nc.scalar.copy(dbc[:, jc * 512:(jc + 1) * 512], bc_ps[:])
                dbcs[lb] = dbc

            # first two broadcasts as soon as d_sb exists
            bcast(0)
            bcast(1)

            # h -> [i, l] layout via PE transposes of [16, 128] blocks
            def h_transpose(ic):
                loc = ic * 128
                tp = ptp.tile([128, LH], F32, name="tp", tag="tp")
                nc.tensor.transpose(
                    tp[:], h_lT[64:64 + LH, loc:loc + 128],
                    id_sb[64:64 + LH, :])
                nc.vector.tensor_copy(out=h_all[:, ic, :], in_=tp[:])

            for ic in range(4):
                h_transpose(ic)

            def add_one(ot, lb, ic):
                nc.vector.tensor_scalar_add(ot[:, ic, :], dbcs[lb][:],
                                            h_all[:, ic, lb:lb + 1])

            # label 0, first half: launches the output stream before the
            # h j-half-1 path resolves
            ot0 = opool.tile([128, ICH, S], F16, name="ot", tag="ot")
            for ic in range(4):
                add_one(ot0, 0, ic)
            nc.sync.dma_start(out=out_v[0, :, 0:4, :], in_=ot0[:, 0:4, :])

            # h j-half 1, also @ group 64 (its bank is sc_b's, its column
            # group reopens once sc_c is evacuated; PE is free by then)
            sc_d = psc.tile([128, 512], F32, name="sc_d", tag="score")
            for k in range(KCH):
                nc.tensor.matmul(
                    sc_d[64:64 + LH, :], wT_sb[:, k, 0:LH],
                    headT_sb[:, 1, k, :],
                    start=(k == 0), stop=(k == KCH - 1),
                    tile_position=(0, 64),
                )
            nc.scalar.copy(h_lT[64:64 + LH, 512:1024], sc_d[64:64 + LH, :])
            for ic in range(4, ICH):
                h_transpose(ic)

            # label 0, second half
            for ic in range(4, ICH):
                add_one(ot0, 0, ic)
            nc.sync.dma_start(out=out_v[0, :, 4:8, :], in_=ot0[:, 4:8, :])
            bcast(2)

            # --- steady output loop --------------------------------------
            for lb in range(1, LH):
                ot = opool.tile([128, ICH, S], F16, name="ot", tag="ot")
                for ic in range(ICH):
                    add_one(ot, lb, ic)
                nc.sync.dma_start(out=out_v[lb, :, :, :], in_=ot[:])
                # broadcasts emitted in label order keep the in-order ACT
                # queue (dbc copies only) two labels ahead of the adds
                if lb + 2 < LH:
                    bcast(lb + 2)
    nc.compile()
    return nc


def kernel(head, dep, label_W, label_b):
    global LAST_RESULTS
    head = np.asarray(head, dtype=np.float32)
    dep = np.asarray(dep, dtype=np.float32)
    label_W = np.asarray(label_W, dtype=np.float32)
    label_b = np.asarray(label_b, dtype=np.float32)

    def pack_inp(x):  # [S, D] f32 -> [128, 2, KCH, 512] f16, d = k*128+p
        xT = np.ascontiguousarray(x.T).astype(np.float16)   # [D, S]
        return np.ascontiguousarray(
            xT.reshape(KCH, 128, 2, 512).transpose(1, 2, 0, 3))

    headP = [pack_inp(head[b]) for b in range(B)]
    depP = [pack_inp(dep[b]) for b in range(B)]
    whT = label_W[:, :D].T.astype(np.float16)   # [D, L]
    wdT = label_W[:, D:].T.astype(np.float16)   # [D, L]

    # one-hot selector sel[k, l*128 + p] = (k == l), replicated at
    # partition groups 0 and 32 (one per j-half broadcast matmul)
    sel = np.zeros((48, LH * 128), dtype=np.float16)
    for lb in range(LH):
        sel[lb, lb * 128:(lb + 1) * 128] = 1.0
    sel[32:48] = sel[0:LH]
    # identity block for the h transposes at partition group 64
    idm = np.zeros((80, LH), dtype=np.float32)
    idm[64:80] = np.eye(LH, dtype=np.float32)

    in_maps = []
    for c in range(NCORES):
        b, lh = divmod(c, 2)
        ls = slice(lh * LH, (lh + 1) * LH)
        bc = np.zeros((48, 1), dtype=np.float32)
        bc[0:LH, 0] = label_b[ls]
        bc[32:48, 0] = label_b[ls]
        wt = np.concatenate([whT[:, ls], wdT[:, ls]], axis=1)  # [D, 32]
        in_maps.append({
            "headT": headP[b],
            "depT": depP[b],
            "wT": np.ascontiguousarray(wt),
            "bcol": bc,
            "sel": sel,
            "idm": idm,
        })

    if "nc" not in _CACHE:
        _CACHE["nc"] = _build()
    nc = _CACHE["nc"]

    res = run_bass_kernel_spmd(nc, in_maps, core_ids=list(range(NCORES)),
                               trace=TRACE, trace_cores=TRACE_CORES)
    LAST_RESULTS = res

    out = np.empty((B, L, S, S), dtype=np.float32)
    for c in range(NCORES):
        b, lh = divmod(c, 2)
        # device layout [l, p, c, j] with i = c*128 + p -> [l, i, j]
        o = np.asarray(res.results[c]["out"])  # [16, 128, 8, 1024] f16
        o = o.transpose(0, 2, 1, 3).reshape(LH, S, S)
        out[b, lh * LH:(lh + 1) * LH] = o.astype(np.float32)
    return out


# revision 14
# speedup vs baseline: 1.0897x; 1.0897x over previous
"""AffineLabelAttention Trainium2 kernel (v3).

out[b, l, i, j] = W_h[l] @ head[b, i] + W_d[l] @ dep[b, j] + bias[l]

Shapes (hardcoded): head/dep [4, 1024, 768] f32, label_W [32, 1536], label_b [32].
Full output [4, 32, 1024, 1024] f32 (512 MB) -> completely output-DMA-bound.

Sharding over 8 cores: core c handles batch b = c // 2 and label half
lh = c % 2 (16 labels).

The device stores the output in float16 (pointwise rel err <= 2^-11);
the host upcasts during the unshard. 32 MB of output per core is the
whole cost: a single HWDGE queue fans each DMA across all 16 SDMA
engines and sustains ~420 GB/s (measured, near the 435 GB/s SBUF-AXI
fabric ceiling), so one queue IS the roofline. The kernel's only job
is to start that stream as early as possible and never let it starve.

v3 structure (109 us v2 -> target ~99 us):
  1. Inputs are host-cast to f16 AND host-rearranged so every DMA is
     contiguous per partition ([p][jc][k][s] layout): input staging
     runs at line rate instead of ~320 GB/s.
  2. Staging order per ring: dep before head, head j-half 0 before
     j-half 1, split across both HWDGE rings; consts on SWDGE.
  3. PE warm-up chain sized to end right when dep lands, so the HAM
     clock boost (needs ~4 us of sustained duty) arrives BEFORE the
     score matmuls instead of 8 us into them (v2's first-DMA was 27 us
     because the d/h matmuls ran at half clock).
  4. ALL 128 output adds run on DVE (f16 SBUF tensor_scalar = 4x perf
     mode, ~330 ns per [128,1024] tile; ~56% occupancy under the
     stream). ACT only evacuates PSUM (d/h scores, broadcast rows).
  5. Per label one 2 MB DMA on the sync ring (label 0 split 1 MB +
     1 MB, with the ic4-7 adds emitted after the h j-half-1 transposes
     so the first 1 MB launches without waiting on them).

  Notes baked into the structure:
  - walrus/bass: compute-engine operands must start at partition
    0/32/64 (96 is rejected); engines cannot move data across
    partitions (only PE matmul/transpose and DMA can).
  - PSUM is 8 banks x 2KB: score pool 2 + warmup/transpose pool 2 +
    broadcast pool 4.
  - A DMA trigger that cannot get a ring slot stalls its issuing
    engine: ACT issues only 3 input transfers, all before its compute.
  - PSUM operands cap DVE perf modes, so broadcasts are evacuated to
    SBUF f16 by ACT (closest to PSUM) and the adds read SBUF at 4x.
"""

import sys

import numpy as np

if "/opt/trn_rl_repo" not in sys.path:
    sys.path.insert(0, "/opt/trn_rl_repo")

import concourse.bass as bass
import concourse.mybir as mybir
from concourse import bacc
from concourse.bass_utils import run_bass_kernel_spmd
from concourse.tile import TileContext, add_dep_helper

B, S, D, L = 4, 1024, 768, 32
NCORES = 8
LH = L // 2          # labels per core (16)
KCH = D // 128       # contraction chunks (6)
ICH = S // 128       # i chunks (8)
F32 = mybir.dt.float32
F16 = mybir.dt.float16
WU_N = 12            # PE warm-up matmuls before the score streams

# knobs for test harness
TRACE = False
TRACE_CORES = None
LAST_RESULTS = None

_CACHE = {}


def _build():
    nc = bacc.Bacc("TRN2", target_bir_lowering=False, debug=False)
    # inputs pre-rearranged on host: [partition, jc, k, s'] where
    # d = k*128 + p contracts and j (or i) = jc*512 + s'
    headT = nc.dram_tensor("headT", [128, 2, KCH, 512], F16,
                           kind="ExternalInput")
    depT = nc.dram_tensor("depT", [128, 2, KCH, 512], F16,
                          kind="ExternalInput")
    # packed label weights: cols 0:16 = W_h slice, 16:32 = W_d slice
    wT = nc.dram_tensor("wT", [D, 2 * LH], F16, kind="ExternalInput")
    # bias replicated at partition groups 0 and 32 (one per j-half)
    bcol = nc.dram_tensor("bcol", [48, 1], F32, kind="ExternalInput")
    # one-hot row selectors, replicated at partition groups 0 and 32
    sel = nc.dram_tensor("sel", [48, LH * 128], F16, kind="ExternalInput")
    # identity block for h transposes at partition group 64
    idm = nc.dram_tensor("idm", [80, LH], F32, kind="ExternalInput")
    # [l, p, c, j]: row i = c*128 + p of label l lives at out[l, p, c, :]
    out = nc.dram_tensor("out", [LH, 128, ICH, S], F16, kind="ExternalOutput")
    out_v = out[:]

    headT_f = headT[:]
    depT_f = depT[:]
    wT_f = wT[:].rearrange("(k p) l -> p k l", p=128)         # [128, 6, 32]

    with TileContext(nc) as tc:
        with (
            tc.tile_pool(name="const", bufs=1) as cpool,
            tc.tile_pool(name="outp", bufs=4) as opool,
            tc.tile_pool(name="bcast", bufs=16) as bpool,
            tc.tile_pool(name="psum_sc", bufs=2, space="PSUM") as psc,
            tc.tile_pool(name="psum_tp", bufs=2, space="PSUM") as ptp,
            tc.tile_pool(name="psum_bc", bufs=4, space="PSUM") as pbc,
        ):
            depT_sb = cpool.tile([128, 2, KCH, 512], F16)
            headT_sb = cpool.tile([128, 2, KCH, 512], F16)
            wT_sb = cpool.tile([128, KCH, 2 * LH], F16)
            b_col = cpool.tile([48, 1], F32)
            sel_sb = cpool.tile([48, LH * 128], F16)
            id_sb = cpool.tile([80, LH], F32)
            h_lT = cpool.tile([128, S], F32)     # h scores [l, i] @ parts 64:80
            h_all = cpool.tile([128, ICH, LH], F32)  # h scores, [i, l] layout
            d_sb = cpool.tile([48, S], F16)      # d+bias: jc0 @ 0:16, jc1 @ 32:48
            wu_w = cpool.tile([128, LH], F16)    # PE warm-up operands
            wu_x = cpool.tile([128, 512], F16)

            # Warm-up operand memsets first so DVE clears them at t~0 and
            # the PE warm-up chain starts immediately.
            nc.vector.memset(wu_w[:], 0.0)
            nc.vector.memset(wu_x[:], 0.0)

            # --- input staging -------------------------------------------
            # sync ring:   dep-jc0, head-jc0 k0-2, head-jc1 k0-2,
            #              then all output DMAs (strict FIFO per ring).
            # scalar ring: dep-jc1, head-jc0 k3-5, head-jc1 k3-5 (ACT
            #              issues these before any of its compute).
            # gpsimd ring: weights + consts via SWDGE (tiny; lands ~9 us,
            #              keeps the HWDGE rings free for dep/head).
            nc.sync.dma_start(out=depT_sb[:, 0], in_=depT_f[:, 0])
            nc.scalar.dma_start(out=depT_sb[:, 1], in_=depT_f[:, 1])
            nc.sync.dma_start(out=headT_sb[:, 0, 0:3], in_=headT_f[:, 0, 0:3])
            nc.scalar.dma_start(out=headT_sb[:, 0, 3:6],
                                in_=headT_f[:, 0, 3:6])
            nc.sync.dma_start(out=headT_sb[:, 1, 0:3], in_=headT_f[:, 1, 0:3])
            nc.scalar.dma_start(out=headT_sb[:, 1, 3:6],
                                in_=headT_f[:, 1, 3:6])
            nc.gpsimd.dma_start(out=wT_sb[:], in_=wT_f[:])
            nc.gpsimd.dma_start(out=sel_sb[:], in_=sel[:])
            nc.gpsimd.dma_start(out=id_sb[:], in_=idm[:])
            nc.gpsimd.dma_start(out=b_col[:], in_=bcol[:])

            # PE warm-up (builds HAM clock duty while inputs stream in;
            # sized to end about when dep lands). Lives in the transpose
            # pool so it never blocks the score psums.
            wu_ps = ptp.tile([128, 512], F32, name="wu", tag="tp")
            for _ in range(WU_N):
                nc.tensor.matmul(wu_ps[0:LH, :], wu_w[:], wu_x[:],
                                 start=True, stop=True)

            # d scores: two concurrent column-group streams (jc0 @ group 0,
            # jc1 @ group 32), issue-interleaved so the array pipelines the
            # LdWeights of one group under the matmul of the other.
            sc_a = psc.tile([128, 512], F32, name="sc_a", tag="score")
            sc_b = psc.tile([128, 512], F32, name="sc_b", tag="score")
            for k in range(KCH):
                nc.tensor.matmul(
                    sc_a[0:LH, :], wT_sb[:, k, LH:2 * LH],
                    depT_sb[:, 0, k, :],
                    start=(k == 0), stop=(k == KCH - 1),
                    tile_position=(0, 0),
                )
                nc.tensor.matmul(
                    sc_b[32:32 + LH, :], wT_sb[:, k, LH:2 * LH],
                    depT_sb[:, 1, k, :],
                    start=(k == 0), stop=(k == KCH - 1),
                    tile_position=(0, 32),
                )

            # d evacuation (+bias) on ACT (fastest PSUM reader), f16 out
            nc.scalar.add(d_sb[0:LH, 0:512], sc_a[0:LH, :], b_col[0:LH, :])
            nc.scalar.add(d_sb[32:32 + LH, 512:1024],
                          sc_b[32:32 + LH, :], b_col[32:32 + LH, :])

            # h j-half 0 @ group 64 (needs head-jc0 only)
            sc_c = psc.tile([128, 512], F32, name="sc_c", tag="score")
            for k in range(KCH):
                nc.tensor.matmul(
                    sc_c[64:64 + LH, :], wT_sb[:, k, 0:LH],
                    headT_sb[:, 0, k, :],
                    start=(k == 0), stop=(k == KCH - 1),
                    tile_position=(0, 64),
                )
            nc.scalar.copy(h_lT[64:64 + LH, 0:512], sc_c[64:64 + LH, :])

            dbcs = {}

            def bcast(lb):
                # replicate d row lb across 128 partitions: one-hot selector
                # matmuls (f16 exact). Result evacuated to a persistent f16
                # SBUF tile so the adds run in DVE 4x mode.
                dbc = bpool.tile([128, S], F16, name="dbc", tag="dbc")
                for jc in range(2):
                    p0 = 32 * jc
                    bc_ps = pbc.tile([128, 512], F32, name="bc", tag="bc")
                    nc.tensor.matmul(
                        bc_ps[:],
                        sel_sb[p0:p0 + LH, lb * 128:(lb + 1) * 128],
                        d_sb[p0:p0 + LH, jc * 512:(jc + 1) * 512],
                        start=True, stop=True,
                    )
                    nc.scalar.copy(dbc[:, jc * 512:(jc + 1) * 512], bc_ps[:])
                dbcs[lb] = dbc

            # first two broadcasts as soon as d_sb exists
            bcast(0)
            bcast(1)

            # h -> [i, l] layout via PE transposes of [16, 128] blocks
            def h_transpose(ic):
                loc = ic * 128
                tp = ptp.tile([128, LH], F32, name="tp", tag="tp")
                nc.tensor.transpose(
                    tp[:], h_lT[64:64 + LH, loc:loc + 128],
                    id_sb[64:64 + LH, :])
                nc.vector.tensor_copy(out=h_all[:, ic, :], in_=tp[:])

            for ic in range(4):
                h_transpose(ic)

            def add_one(ot, lb, ic, on_dve):
                scal = h_all[:, ic, lb:lb + 1]
                if on_dve:
                    nc.vector.tensor_scalar_add(ot[:, ic, :], dbcs[lb][:],
                                                scal)
                else:
                    nc.scalar.add(ot[:, ic, :], dbcs[lb][:], scal)

            # label 0, first half: launches the output stream before the
            # h j-half-1 path resolves (ic2-3 on ACT, in parallel with DVE)
            ot0 = opool.tile([128, ICH, S], F16, name="ot", tag="ot")
            for ic in range(4):
                add_one(ot0, 0, ic, on_dve=(ic < 2))
            nc.sync.dma_start(out=out_v[0, :, 0:4, :], in_=ot0[:, 0:4, :])

            # h j-half 1, also @ group 64 (its bank is sc_b's, its column
            # group reopens once sc_c is evacuated; PE is free by then)
            sc_d = psc.tile([128, 512], F32, name="sc_d", tag="score")
            for k in range(KCH):
                nc.tensor.matmul(
                    sc_d[64:64 + LH, :], wT_sb[:, k, 0:LH],
                    headT_sb[:, 1, k, :],
                    start=(k == 0), stop=(k == KCH - 1),
                    tile_position=(0, 64),
                )
            nc.scalar.copy(h_lT[64:64 + LH, 512:1024], sc_d[64:64 + LH, :])
            for ic in range(4, ICH):
                h_transpose(ic)

            # label 0, second half
            for ic in range(4, ICH):
                add_one(ot0, 0, ic, on_dve=(ic < 6))
            nc.sync.dma_start(out=out_v[0, :, 4:8, :], in_=ot0[:, 4:8, :])
            bcast(2)

            # --- steady output loop --------------------------------------
            # DVE takes 6 adds, ACT 2 adds + 2 dbc copies per label: both
            # finish just under the 2 MB DMA drain time, so trigger
            # admission tracks the drain rate and the 16 SDMA engines stay
            # in lockstep (bursty admission lets the slow engine 15 build
            # a private backlog that drains alone as a 13 us tail).
            for lb in range(1, LH):
                ot = opool.tile([128, ICH, S], F16, name="ot", tag="ot")
                for ic in range(ICH):
                    add_one(ot, lb, ic, on_dve=(ic < 6))
                nc.sync.dma_start(out=out_v[lb, :, :, :], in_=ot[:])
                # broadcasts emitted AFTER each label's adds: on the
                # in-order ACT queue the dbc copies must sit behind this
                # label's adds, or every label gates on the next label's
                # broadcast evacuation
                if lb + 2 < LH:
                    bcast(lb + 2)
    nc.compile()
    return nc


def kernel(head, dep, label_W, label_b):
    global LAST_RESULTS
    head = np.asarray(head, dtype=np.float32)
    dep = np.asarray(dep, dtype=np.float32)
    label_W = np.asarray(label_W, dtype=np.float32)
    label_b = np.asarray(label_b, dtype=np.float32)

    def pack_inp(x):  # [S, D] f32 -> [128, 2, KCH, 512] f16, d = k*128+p
        xT = np.ascontiguousarray(x.T).astype(np.float16)   # [D, S]
        return np.ascontiguousarray(
            xT.reshape(KCH, 128, 2, 512).transpose(1, 2, 0, 3))

    headP = [pack_inp(head[b]) for b in range(B)]
    depP = [pack_inp(dep[b]) for b in range(B)]
    whT = label_W[:, :D].T.astype(np.float16)   # [D, L]
    wdT = label_W[:, D:].T.astype(np.float16)   # [D, L]

    # one-hot selector sel[k, l*128 + p] = (k == l), replicated at
    # partition groups 0 and 32 (one per j-half broadcast matmul)
    sel = np.zeros((48, LH * 128), dtype=np.float16)
    for lb in range(LH):
        sel[lb, lb * 128:(lb + 1) * 128] = 1.0
    sel[32:48] = sel[0:LH]
    # identity block for the h transposes at partition group 64
    idm = np.zeros((80, LH), dtype=np.float32)
    idm[64:80] = np.eye(LH, dtype=np.float32)

    in_maps = []
    for c in range(NCORES):
        b, lh = divmod(c, 2)
        ls = slice(lh * LH, (lh + 1) * LH)
        bc = np.zeros((48, 1), dtype=np.float32)
        bc[0:LH, 0] = label_b[ls]
        bc[32:48, 0] = label_b[ls]
        wt = np.concatenate([whT[:, ls], wdT[:, ls]], axis=1)  # [D, 32]
        in_maps.append({
            "headT": headP[b],
            "depT": depP[b],
            "wT": np.ascontiguousarray(wt),
            "bcol": bc,
            "sel": sel,
            "idm": idm,
        })

    if "nc" not in _CACHE:
        _CACHE["nc"] = _build()
    nc = _CACHE["nc"]

    res = run_bass_kernel_spmd(nc, in_maps, core_ids=list(range(NCORES)),
                               trace=TRACE, trace_cores=TRACE_CORES)
    LAST_RESULTS = res

    out = np.empty((B, L, S, S), dtype=np.float32)
    for c in range(NCORES):
        b, lh = divmod(c, 2)
        # device layout [l, p, c, j] with i = c*128 + p -> [l, i, j]
        o = np.asarray(res.results[c]["out"])  # [16, 128, 8, 1024] f16
        o = o.transpose(0, 2, 1, 3).reshape(LH, S, S)
        out[b, lh * LH:(lh + 1) * LH] = o.astype(np.float32)
    return out


# revision 16
# speedup vs baseline: 1.1681x; 1.0719x over previous
"""AffineLabelAttention Trainium2 kernel (v5).

out[b, l, i, j] = W_h[l] @ head[b, i] + W_d[l] @ dep[b, j] + bias[l]

Shapes (hardcoded): head/dep [4, 1024, 768] f32, label_W [32, 1536], label_b [32].
Full output [4, 32, 1024, 1024] f32 (512 MB) -> completely output-DMA-bound.

Sharding over 8 cores: core c handles batch b = c // 2 and label half
lh = c % 2 (16 labels).

The device stores the output in float16 (pointwise rel err <= 2^-11);
the host upcasts during the unshard. 32 MB of output per core is the
whole cost: a single HWDGE queue fans each DMA across all 16 SDMA
engines and sustains ~420 GB/s (measured), so one queue IS the
roofline. The kernel's only job is to start that stream as early as
possible and never let it starve.

Structure (what profiling showed matters):
  1. Inputs host-cast to f16 and host-rearranged to a per-partition
     contiguous [p][jc][k][s] layout; every PE matmul is 1-pass f16.
  2. NO SWDGE (gpsimd) DMAs: the Q7 software descriptor generation for
     small constants sprays hundreds of tiny ring descriptors whose
     SBUF-port traffic stalls the HWDGE SDMA engines for ~6 us right
     in the middle of input staging. All constants (weights, one-hot
     selectors, transpose identity, bias) are packed into ONE
     zero-padded [128, 2304] f16 tensor, the sync ring's first
     transfer (~0.6 us).
  3. Staging order: consts, then dep (both HWDGE rings), then head
     j-half 0, then j-half 1. The PE warm-up chain is sized to end
     right as dep lands: the HAM clock boost needs ~3-4 us of
     CONTINUOUS PE duty, and any idle gap before the score matmuls
     drops the whole score phase to half clock (~8 us of extra start
     latency).
  4. Per label: d-row broadcast via one-hot PE matmuls into PSUM,
     evacuated by ACT to persistent f16 SBUF tiles; adds run 6 on DVE
     (f16 SBUF tensor_scalar = 4x perf mode, ~480 ns per [128,1024]
     tile) and 2 on ACT. Both engines finish just under the 2 MB DMA
     drain time, so trigger admission tracks the drain rate and the
     16 SDMA engines stay in lockstep (bursty admission lets the slow
     engine 15 build a private backlog that drains alone as a 6-13 us
     tail after the last trigger).
  5. Label 0 is split 1 MB + 1 MB with its first-group adds split
     DVE/ACT so the stream launches before the h j-half-1 path
     (matmul + transposes) resolves.

  Notes baked into the structure:
  - walrus/bass: compute-engine operands must start at partition
    0/32/64 (96 is rejected); engines cannot move data across
    partitions (only PE matmul/transpose and DMA can).
  - PSUM is 8 banks x 2KB: score pool 2 + warmup/transpose pool 2 +
    broadcast pool 4.
  - A DMA trigger that cannot get a ring slot stalls its issuing
    engine: ACT issues only 3 input transfers, all before its compute.
  - PSUM operands cap DVE perf modes, so broadcasts are evacuated to
    SBUF f16 by ACT (closest to PSUM) and the adds read SBUF at 4x.
"""

import sys

import numpy as np

if "/opt/trn_rl_repo" not in sys.path:
    sys.path.insert(0, "/opt/trn_rl_repo")

import concourse.bass as bass
import concourse.mybir as mybir
from concourse import bacc
from concourse.bass_utils import run_bass_kernel_spmd
from concourse.tile import TileContext, add_dep_helper

B, S, D, L = 4, 1024, 768, 32
NCORES = 8
LH = L // 2          # labels per core (16)
KCH = D // 128       # contraction chunks (6)
ICH = S // 128       # i chunks (8)
F32 = mybir.dt.float32
F16 = mybir.dt.float16
WU_N = 12            # PE warm-up matmuls before the score streams

# packed-constant column offsets (f16 columns)
PK_W = 0             # wT:  cols k*32 + (0:16)=W_h, (16:32)=W_d, 6 k-chunks
PK_SEL = 192         # sel: one-hot rows, partition groups 0 and 32
PK_ID = PK_SEL + LH * 128   # 2240: identity eye at partitions 64:80
PK_B = PK_ID + LH           # 2256: bias column at partition groups 0/32
PK_N = 2304          # padded total

# knobs for test harness
TRACE = False
TRACE_CORES = None
LAST_RESULTS = None

_CACHE = {}


def _build():
    nc = bacc.Bacc("TRN2", target_bir_lowering=False, debug=False)
    # inputs pre-rearranged on host: [partition, jc, k, s'] where
    # d = k*128 + p contracts and j (or i) = jc*512 + s'
    headT = nc.dram_tensor("headT", [128, 2, KCH, 512], F16,
                           kind="ExternalInput")
    depT = nc.dram_tensor("depT", [128, 2, KCH, 512], F16,
                          kind="ExternalInput")
    pkd = nc.dram_tensor("pk", [128, PK_N], F16, kind="ExternalInput")
    # [l, p, c, j]: row i = c*128 + p of label l lives at out[l, p, c, :]
    out = nc.dram_tensor("out", [LH, 128, ICH, S], F16, kind="ExternalOutput")
    out_v = out[:]

    headT_f = headT[:]
    depT_f = depT[:]

    with TileContext(nc) as tc:
        with (
            tc.tile_pool(name="const", bufs=1) as cpool,
            tc.tile_pool(name="outp", bufs=4) as opool,
            tc.tile_pool(name="bcast", bufs=16) as bpool,
            tc.tile_pool(name="psum_sc", bufs=2, space="PSUM") as psc,
            tc.tile_pool(name="psum_tp", bufs=2, space="PSUM") as ptp,
            tc.tile_pool(name="psum_bc", bufs=4, space="PSUM") as pbc,
        ):
            depT_sb = cpool.tile([128, 2, KCH, 512], F16)
            headT_sb = cpool.tile([128, 2, KCH, 512], F16)
            pk = cpool.tile([128, PK_N], F16)
            h_lT = cpool.tile([128, S], F16)     # h scores [l, i] @ parts 64:80
            h_all = cpool.tile([128, ICH, LH], F32)  # h scores, [i, l] layout
            d_sb = cpool.tile([48, S], F16)      # d+bias: jc0 @ 0:16, jc1 @ 32:48
            wu_w = cpool.tile([128, LH], F16)    # PE warm-up operands
            wu_x = cpool.tile([128, 512], F16)

            def w_h(k):
                return pk[:, PK_W + k * 32:PK_W + k * 32 + LH]

            def w_d(k):
                return pk[:, PK_W + k * 32 + LH:PK_W + k * 32 + 2 * LH]

            def sel_v(jc, lb):
                p0 = 32 * jc
                return pk[p0:p0 + LH, PK_SEL + lb * 128:PK_SEL + (lb + 1) * 128]

            id_v = pk[64:64 + LH, PK_ID:PK_ID + LH]
            b_col = pk[0:48, PK_B:PK_B + 1]

            # Warm-up operand memsets first so DVE clears them at t~0 and
            # the PE warm-up chain starts immediately.
            nc.vector.memset(wu_w[:], 0.0)
            nc.vector.memset(wu_x[:], 0.0)

            # --- input staging -------------------------------------------
            # sync ring:   consts, dep-jc0, head-jc0 k0-2, head-jc1 k0-2,
            #              then all output DMAs (strict FIFO per ring).
            # scalar ring: dep-jc1, head-jc0 k3-5, head-jc1 k3-5 (ACT
            #              issues these before any of its compute).
            nc.sync.dma_start(out=pk[:], in_=pkd[:])
            nc.sync.dma_start(out=depT_sb[:, 0], in_=depT_f[:, 0])
            nc.scalar.dma_start(out=depT_sb[:, 1], in_=depT_f[:, 1])
            nc.sync.dma_start(out=headT_sb[:, 0, 0:3], in_=headT_f[:, 0, 0:3])
            nc.scalar.dma_start(out=headT_sb[:, 0, 3:6],
                                in_=headT_f[:, 0, 3:6])
            nc.sync.dma_start(out=headT_sb[:, 1, 0:3], in_=headT_f[:, 1, 0:3])
            nc.scalar.dma_start(out=headT_sb[:, 1, 3:6],
                                in_=headT_f[:, 1, 3:6])

            # PE warm-up (builds HAM clock duty while inputs stream in;
            # sized to end about when dep lands). Lives in the transpose
            # pool so it never blocks the score psums.
            wu_ps = ptp.tile([128, 512], F32, name="wu", tag="tp")
            for _ in range(WU_N):
                nc.tensor.matmul(wu_ps[0:LH, :], wu_w[:], wu_x[:],
                                 start=True, stop=True)

            # d scores: two concurrent column-group streams (jc0 @ group 0,
            # jc1 @ group 32), issue-interleaved so the array pipelines the
            # LdWeights of one group under the matmul of the other.
            sc_a = psc.tile([128, 512], F32, name="sc_a", tag="score")
            sc_b = psc.tile([128, 512], F32, name="sc_b", tag="score")
            for k in range(KCH):
                nc.tensor.matmul(
                    sc_a[0:LH, :], w_d(k), depT_sb[:, 0, k, :],
                    start=(k == 0), stop=(k == KCH - 1),
                    tile_position=(0, 0),
                )
                nc.tensor.matmul(
                    sc_b[32:32 + LH, :], w_d(k), depT_sb[:, 1, k, :],
                    start=(k == 0), stop=(k == KCH - 1),
                    tile_position=(0, 32),
                )

            # d evacuation (+bias) on ACT (fastest PSUM reader), f16 out
            nc.scalar.add(d_sb[0:LH, 0:512], sc_a[0:LH, :], b_col[0:LH, :])
            nc.scalar.add(d_sb[32:32 + LH, 512:1024],
                          sc_b[32:32 + LH, :], b_col[32:32 + LH, :])

            # h j-half 0 @ group 64 (needs head-jc0 only)
            sc_c = psc.tile([128, 512], F32, name="sc_c", tag="score")
            for k in range(KCH):
                nc.tensor.matmul(
                    sc_c[64:64 + LH, :], w_h(k), headT_sb[:, 0, k, :],
                    start=(k == 0), stop=(k == KCH - 1),
                    tile_position=(0, 64),
                )
            nc.scalar.copy(h_lT[64:64 + LH, 0:512], sc_c[64:64 + LH, :])

            dbcs = {}

            def bcast(lb):
                # replicate d row lb across 128 partitions: one-hot selector
                # matmuls (f16 exact). Result evacuated to a persistent f16
                # SBUF tile so the adds run in DVE 4x mode.
                dbc = bpool.tile([128, S], F16, name="dbc", tag="dbc")
                for jc in range(2):
                    bc_ps = pbc.tile([128, 512], F32, name="bc", tag="bc")
                    nc.tensor.matmul(
                        bc_ps[:], sel_v(jc, lb),
                        d_sb[32 * jc:32 * jc + LH,
                             jc * 512:(jc + 1) * 512],
                        start=True, stop=True,
                    )
                    nc.scalar.copy(dbc[:, jc * 512:(jc + 1) * 512], bc_ps[:])
                dbcs[lb] = dbc

            # first two broadcasts as soon as d_sb exists
            bcast(0)
            bcast(1)

            # h -> [i, l] layout via PE transposes of [16, 128] blocks
            def h_transpose(ic):
                loc = ic * 128
                tp = ptp.tile([128, LH], F16, name="tp", tag="tp")
                nc.tensor.transpose(
                    tp[:], h_lT[64:64 + LH, loc:loc + 128], id_v)
                nc.vector.tensor_copy(out=h_all[:, ic, :], in_=tp[:])

            for ic in range(4):
                h_transpose(ic)

            def add_one(ot, lb, ic, on_dve):
                scal = h_all[:, ic, lb:lb + 1]
                if on_dve:
                    nc.vector.tensor_scalar_add(ot[:, ic, :], dbcs[lb][:],
                                                scal)
                else:
                    nc.scalar.add(ot[:, ic, :], dbcs[lb][:], scal)

            # label 0, first half: launches the output stream before the
            # h j-half-1 path resolves (ic2-3 on ACT, in parallel with DVE)
            ot0 = opool.tile([128, ICH, S], F16, name="ot", tag="ot")
            for ic in range(4):
                add_one(ot0, 0, ic, on_dve=(ic < 2))
            nc.sync.dma_start(out=out_v[0, :, 0:4, :], in_=ot0[:, 0:4, :])

            # h j-half 1, also @ group 64 (its bank is sc_b's, its column
            # group reopens once sc_c is evacuated; PE is free by then)
            sc_d = psc.tile([128, 512], F32, name="sc_d", tag="score")
            for k in range(KCH):
                nc.tensor.matmul(
                    sc_d[64:64 + LH, :], w_h(k), headT_sb[:, 1, k, :],
                    start=(k == 0), stop=(k == KCH - 1),
                    tile_position=(0, 64),
                )
            nc.scalar.copy(h_lT[64:64 + LH, 512:1024], sc_d[64:64 + LH, :])
            for ic in range(4, ICH):
                h_transpose(ic)

            # label 0, second half
            for ic in range(4, ICH):
                add_one(ot0, 0, ic, on_dve=(ic < 6))
            nc.sync.dma_start(out=out_v[0, :, 4:8, :], in_=ot0[:, 4:8, :])
            bcast(2)

            # --- steady output loop --------------------------------------
            for lb in range(1, LH):
                ot = opool.tile([128, ICH, S], F16, name="ot", tag="ot")
                for ic in range(ICH):
                    add_one(ot, lb, ic, on_dve=(ic < 6))
                nc.sync.dma_start(out=out_v[lb, :, :, :], in_=ot[:])
                # broadcasts emitted AFTER each label's adds: on the
                # in-order ACT queue the dbc copies must sit behind this
                # label's adds, or every label gates on the next label's
                # broadcast evacuation
                if lb + 2 < LH:
                    bcast(lb + 2)
    nc.compile()
    return nc


def kernel(head, dep, label_W, label_b):
    global LAST_RESULTS
    head = np.asarray(head, dtype=np.float32)
    dep = np.asarray(dep, dtype=np.float32)
    label_W = np.asarray(label_W, dtype=np.float32)
    label_b = np.asarray(label_b, dtype=np.float32)

    def pack_inp(x):  # [S, D] f32 -> [128, 2, KCH, 512] f16, d = k*128+p
        xT = np.ascontiguousarray(x.T).astype(np.float16)   # [D, S]
        return np.ascontiguousarray(
            xT.reshape(KCH, 128, 2, 512).transpose(1, 2, 0, 3))

    headP = [pack_inp(head[b]) for b in range(B)]
    depP = [pack_inp(dep[b]) for b in range(B)]
    whT = label_W[:, :D].T.astype(np.float16)   # [D, L]
    wdT = label_W[:, D:].T.astype(np.float16)   # [D, L]

    in_maps = []
    for c in range(NCORES):
        b, lh = divmod(c, 2)
        ls = slice(lh * LH, (lh + 1) * LH)
        pack = np.zeros((128, PK_N), dtype=np.float16)
        # weights: pack[p, k*32 + c] = W[d = k*128 + p, label c]
        for k in range(KCH):
            pack[:, PK_W + k * 32:PK_W + k * 32 + LH] = \
                whT[k * 128:(k + 1) * 128, ls]
            pack[:, PK_W + k * 32 + LH:PK_W + k * 32 + 2 * LH] = \
                wdT[k * 128:(k + 1) * 128, ls]
        # one-hot selectors at partition groups 0 and 32
        for lb in range(LH):
            pack[lb, PK_SEL + lb * 128:PK_SEL + (lb + 1) * 128] = 1.0
            pack[32 + lb, PK_SEL + lb * 128:PK_SEL + (lb + 1) * 128] = 1.0
        # transpose identity at partition group 64
        pack[64:64 + LH, PK_ID:PK_ID + LH] = np.eye(LH, dtype=np.float16)
        # bias column at partition groups 0 and 32
        pack[0:LH, PK_B] = label_b[ls]
        pack[32:48, PK_B] = label_b[ls]
        in_maps.append({
            "headT": headP[b],
            "depT": depP[b],
            "pk": pack,
        })

    if "nc" not in _CACHE:
        _CACHE["nc"] = _build()
    nc = _CACHE["nc"]

    res = run_bass_kernel_spmd(nc, in_maps, core_ids=list(range(NCORES)),
                               trace=TRACE, trace_cores=TRACE_CORES)
    LAST_RESULTS = res

    out = np.empty((B, L, S, S), dtype=np.float32)
    for c in range(NCORES):
        b, lh = divmod(c, 2)
        # device layout [l, p, c, j] with i = c*128 + p -> [l, i, j]
        o = np.asarray(res.results[c]["out"])  # [16, 128, 8, 1024] f16
        o = o.transpose(0, 2, 1, 3).reshape(LH, S, S)
        out[b, lh * LH:(lh + 1) * LH] = o.astype(np.float32)
    return out
